# revision 8
# baseline (speedup 1.0000x reference)
"""Multi-head attention (B=2, S=2048, E=1024, H=16, D=64) on 8 TRN2 cores.

Sharding: tensor-parallel over heads. Core c owns heads {2c, 2c+1}:
  - Q/K/V projections column-sharded (128 cols each per core)
  - attention for the core's 2 heads (both batches)
  - out-projection row-sharded (128 rows of Wo) -> partial [4096,1024] f16
  - host sums the 8 partials and adds bo.

On-chip layout (everything "transposed"):
  - host passes xT [1024, 4096] (E-major, fp16) so the contraction dim
    lands on SBUF partitions with no on-device transpose of x
  - projections produce Q^T, K^T [128, 4096] (h0 dims on partitions 0-63,
    h1 dims on 64-127) and V^T, PE-transposed to token-major V tiles
  - scores are computed transposed: scores^T[kk, q]; BOTH heads' score
    matmuls are emitted back-to-back with stationary/rhs at base
    partitions 0 and 64 -> the PE runs them CONCURRENTLY in disjoint
    row-groups (row tiling), recovering the half-array loss of K=64
  - one exp per key tile covers both heads [128, 2*QCH] on ACT; the
    key-padding mask folds into the exp() per-partition bias
  - attn@V: Vtm column window [V_h0 | ones | V_h1] (shared ones col 64).
    h0 stationary = cols 0:65  -> av0: dims at partitions 0-63, sum at 64
    h1 stationary = cols 1:129 -> av1: sum at partition 63, dims at
    64-127 (partitions 0-62 are don't-care garbage). The normalized
    outputs of both heads land partition-aligned in one YTC [128, M],
    so the out-projection is a single K=128 matmul per tile.

Schedule:
  - phase 1 interleaves batch-0 projections (per-512-token chunk) with
    the first attention pass's key-tile quartets, so attention MMs and
    exps fill the projection phase's DMA-wait gaps and ACT starts early
  - batch-1 projections and ready out-proj tiles are filler units popped
    between attention tiles of later passes
  - normalization is DVE+DMA only and runs one pass late; the tail emits
    the last norm BEFORE draining filler so its DMA bounce overlaps PE
"""

import os
import numpy as np

B, S, E, H, D = 2, 2048, 1024, 16, 64
M = B * S            # 4096 tokens
P = 128              # partitions
NCORES = 8
KC = E // P          # 8 contraction chunks for projections
MCH = 512            # token chunk for projections
QCH = 512            # query chunk for attention
NQC = S // QCH       # 4 query chunks per batch
NKT = S // P         # 16 key tiles per batch
NEG = -1.0e30

LAST_RESULTS = None  # BassKernelResults of the most recent run (for test harness)
_PROGRAM = None


def _build_program():
    import concourse.bass as bass
    import concourse.tile as tile
    from concourse import bacc, mybir
    from concourse.masks import make_identity

    f32 = mybir.dt.float32
    f16 = mybir.dt.float16

    nc = bacc.Bacc(
        "TRN2",
        target_bir_lowering=False,
        debug=False,
        enable_asserts=False,
        num_devices=NCORES,
    )

    xT_d = nc.dram_tensor("xT", (E, M), f16, kind="ExternalInput").ap()
    wq_d = nc.dram_tensor("wq", (P, KC, P), f16, kind="ExternalInput").ap()
    wk_d = nc.dram_tensor("wk", (P, KC, P), f16, kind="ExternalInput").ap()
    wv_d = nc.dram_tensor("wv", (P, KC, P), f16, kind="ExternalInput").ap()
    woc_d = nc.dram_tensor("woc", (P, E), f16, kind="ExternalInput").ap()
    bq_d = nc.dram_tensor("bq", (P, 1), f32, kind="ExternalInput").ap()
    bk_d = nc.dram_tensor("bk", (P, 1), f32, kind="ExternalInput").ap()
    bv_d = nc.dram_tensor("bv", (P, 1), f32, kind="ExternalInput").ap()
    maskT_d = nc.dram_tensor("maskT", (P, B * 16), f32, kind="ExternalInput").ap()
    out_d = nc.dram_tensor("out", (M, E), f16, kind="ExternalOutput").ap()
    rsc_d = nc.dram_tensor("rscratch", (32, QCH), f32, kind="Internal").ap()

    def skip_col_ap(base, inner):
        # [P, inner*2] view over a [P, inner*2+1] window, skipping col inner
        return bass.AP(
            tensor=base.tensor,
            offset=base.offset,
            ap=[list(base.ap[0]), [inner + 1, 2], [1, inner]],
        )

    def pair_ap(base, inner):
        # [P, inner*2] contiguous viewed as [P, 2, inner]
        return bass.AP(
            tensor=base.tensor,
            offset=base.offset,
            ap=[list(base.ap[0]), [inner, 2], [1, inner]],
        )

    with tile.TileContext(nc) as tc:
        with (
            tc.tile_pool(name="consts", bufs=1) as consts,
            tc.tile_pool(name="big", bufs=1) as big,
            tc.tile_pool(name="xt_pool", bufs=8) as xt_pool,
            tc.tile_pool(name="vt_pool", bufs=2) as vt_pool,
            tc.tile_pool(name="pt_pool", bufs=8) as pt_pool,
            tc.tile_pool(name="r_pool", bufs=2) as r_pool,
            tc.tile_pool(name="out_pool", bufs=6) as out_pool,
        ):
            # ---- constants ----
            wq_sb = consts.tile([P, KC, P], f16)
            wk_sb = consts.tile([P, KC, P], f16)
            wv_sb = consts.tile([P, KC, P], f16)
            woc_sb = consts.tile([P, E], f16)
            bq_sb = consts.tile([P, 1], f32)
            bk_sb = consts.tile([P, 1], f32)
            bv_sb = consts.tile([P, 1], f32)
            mask_sb = consts.tile([P, B * 16], f32)
            ident = consts.tile([P, P], f16)
            ones_h = consts.tile([P, M // P], f16)

            # first weight chunk on the fast queue (gates the first matmul);
            # everything else on the SWDGE queue behind it in dependency order
            nc.sync.dma_start(wq_sb[:, 0, :], wq_d[:, 0, :])
            nc.scalar.dma_start(
                wq_sb[:, 1:KC, :], wq_d[:, 1:KC, :]
            )
            nc.gpsimd.dma_start(wk_sb, wk_d)
            nc.gpsimd.dma_start(wv_sb, wv_d)
            nc.gpsimd.dma_start(bq_sb, bq_d)
            nc.gpsimd.dma_start(bk_sb, bk_d)
            nc.gpsimd.dma_start(bv_sb, bv_d)
            nc.gpsimd.dma_start(mask_sb, maskT_d)
            nc.gpsimd.dma_start(woc_sb, woc_d)
            make_identity(nc, ident)
            nc.vector.memset(ones_h, 1.0)

            # ---- big persistent activations ----
            QT = big.tile([P, M], f16)       # Q^T: head-dims on partitions
            KT = big.tile([P, M], f16)
            # token-major V tiles: [tok, mt, V_h0(64) | ones | V_h1(64) | pad]
            Vtm = big.tile([P, M // P, 130], f16)
            YTC = big.tile([P, M], f16)      # both heads' attn output^T

            ones_col = ones_h[:, 0 : M // P].rearrange("p (a b) -> p a b", b=1)
            nc.vector.tensor_copy(Vtm[:, :, D : D + 1], ones_col)

            def copy_vtp(mt, vtp):
                # single strided copy: vtp [128,128] -> Vtm cols {0:64, 65:129}
                dst = skip_col_ap(Vtm[:, mt, 0 : 2 * D + 1], D)
                src = pair_ap(vtp[:, 0 : 2 * D], D)
                nc.vector.tensor_copy(dst, src)

            Exp = mybir.ActivationFunctionType.Exp

            def emit_attn_tile(b, qc, t, av0, av1, sc_tile, pt_tile):
                qsl = bass.ds(b * S + qc * QCH, QCH)
                ksl = bass.ds(b * S + t * P, P)
                bt = b * 16 + t
                sc2 = sc_tile()
                # both heads' score matmuls: stationary+rhs at base
                # partitions 0 / 64 -> concurrent row-tiled execution
                nc.tensor.matmul(
                    sc2[:, 0:QCH], KT[0:D, ksl], QT[0:D, qsl],
                    start=True, stop=True,
                )
                nc.tensor.matmul(
                    sc2[:, QCH : 2 * QCH], KT[D:P, ksl], QT[D:P, qsl],
                    start=True, stop=True,
                )
                pt = pt_tile()
                nc.scalar.activation(
                    pt, sc2, Exp, bias=mask_sb[:, bt : bt + 1], scale=1.0
                )
                nc.tensor.matmul(
                    av0[0 : D + 1, :], Vtm[:, bt, 0 : D + 1], pt[:, 0:QCH],
                    start=(t == 0), stop=(t == NKT - 1),
                )
                nc.tensor.matmul(
                    av1, Vtm[:, bt, 1 : 2 * D + 1], pt[:, QCH : 2 * QCH],
                    start=(t == 0), stop=(t == NKT - 1),
                )

            def stage_avs(b, qc, av0, av1, pending):
                av0_sb = r_pool.tile(
                    [D + 1, QCH], f32, tag="avsb", bufs=6, name="av0sb"
                )
                nc.vector.tensor_copy(av0_sb, av0[0 : D + 1, :])
                av1_sb = r_pool.tile(
                    [P, QCH], f32, tag="avsb1", bufs=3, name="av1sb"
                )
                # PSUM engine reads need 32-aligned base partitions: stage
                # [32:64] (sum_h1 rides at 63) and [64:128] (h1 dims)
                nc.vector.tensor_copy(av1_sb[32:D, :], av1[32:D, :])
                nc.vector.tensor_copy(av1_sb[D:P, :], av1[D:P, :])
                pending.append((b, qc, av0_sb, av1_sb))

            norm_idx = [0]

            def emit_norm(b, qc, av0_sb, av1_sb):
                qsl = bass.ds(b * S + qc * QCH, QCH)
                # partition-broadcast the raw sums [1,512] -> halves of a
                # [128,512] tile via DRAM bounce (SBUF-source DMAs cannot
                # have a zero partition step), then one reciprocal at base
                # partition 0 (custom-DVE approx ops misbehave at base 64)
                ni = norm_idx[0]
                norm_idx[0] += 2
                nc.sync.dma_start(rsc_d[ni, :], av0_sb[D : D + 1, :])
                nc.sync.dma_start(rsc_d[ni + 1, :], av1_sb[D - 1 : D, :])
                sb = r_pool.tile([P, QCH], f32, tag="sb", bufs=4, name="sb")
                for hh in range(2):
                    src = rsc_d[ni + hh : ni + hh + 1, :]
                    src_b = bass.AP(
                        tensor=src.tensor,
                        offset=src.offset,
                        ap=[[0, D]] + [list(x) for x in src.ap[1:]],
                    )
                    nc.sync.dma_start(sb[hh * D : (hh + 1) * D, :], src_b)
                rbs = r_pool.tile([P, QCH], f32, tag="rbs", bufs=4, name="rbs")
                rsc2 = r_pool.tile([P, QCH], f32, tag="rsc2", name="rsc2")
                nc.vector.reciprocal_approx_accurate(rbs, sb, rsc2)
                nc.vector.tensor_mul(YTC[0:D, qsl], av0_sb[0:D, :], rbs[0:D, :])
                nc.vector.tensor_mul(YTC[D:P, qsl], av1_sb[D:P, :], rbs[D:P, :])

            # ---- phase 1: batch-0 projections interleaved with attention
            # pass (0,0): proj chunk mc feeds key tiles 4mc..4mc+3 ----
            with tc.tile_pool(name="psum_p1", bufs=2, space="PSUM") as psum_p1:
                p1_pending = []
                av0_p = psum_p1.tile([P, QCH], f32, tag="av0p", bufs=2, name="av0p")
                av1_p = psum_p1.tile([P, QCH], f32, tag="av0p", bufs=2, name="av1p")

                def sc_tile_p1():
                    return psum_p1.tile(
                        [P, 2 * QCH], f32, tag="sc0", bufs=1, name="sc0"
                    )

                def pt_tile():
                    return pt_pool.tile([P, 2 * QCH], f16, tag="pt", name="pt")

                for mc in range(S // MCH):
                    msl = bass.ts(mc, MCH)
                    xts = []
                    for kc in range(KC):
                        xt = xt_pool.tile([P, MCH], f16, tag="xt", name="xt")
                        # ACT is busy with exps; split loads across the sync
                        # and gpsimd queues so DMA dispatch isn't serial
                        eng = nc.sync if kc % 2 == 0 else nc.gpsimd
                        eng.dma_start(xt, xT_d[bass.ts(kc, P), msl])
                        xts.append(xt)
                    # consume sync-queue chunks (even kc) first: the gpsimd
                    # queue drains constants ahead of its odd-kc xt loads
                    kcs = [0, 2, 4, 6, 1, 3, 5, 7]
                    for wi, w_sb, b_sb, dstT in (
                        (0, wq_sb, bq_sb, QT),
                        (1, wk_sb, bk_sb, KT),
                        (2, wv_sb, bv_sb, None),
                    ):
                        pp = psum_p1.tile([P, MCH], f32, tag="p1", name="pp")
                        for i, kc in enumerate(kcs):
                            nc.tensor.matmul(
                                pp, w_sb[:, kc, :], xts[kc],
                                start=(i == 0), stop=(i == KC - 1),
                            )
                        if dstT is not None:
                            nc.vector.tensor_scalar_add(dstT[:, msl], pp, b_sb)
                        else:
                            vt = vt_pool.tile([P, MCH], f16, name="vt")
                            nc.vector.tensor_scalar_add(vt, pp, bv_sb)
                            for j in range(MCH // P):
                                mt = mc * (MCH // P) + j
                                vtp = psum_p1.tile(
                                    [P, P], f16, tag="vtp", bufs=2, name="vtp"
                                )
                                nc.tensor.transpose(
                                    vtp, vt[:, bass.ts(j, P)], ident
                                )
                                copy_vtp(mt, vtp)
                    # attention pass (0,0), key tiles fed by this chunk
                    for t in range(4 * mc, 4 * mc + 4):
                        emit_attn_tile(0, 0, t, av0_p, av1_p, sc_tile_p1, pt_tile)
                stage_avs(0, 0, av0_p, av1_p, p1_pending)

            # ---- phase 2: remaining passes, deferred norm, out-proj ----
            with (
                tc.tile_pool(name="psum_sc", bufs=2, space="PSUM") as psum_sc,
                tc.tile_pool(name="psum_av", bufs=2, space="PSUM") as psum_av,
                tc.tile_pool(name="psum_op", bufs=2, space="PSUM") as psum_op,
            ):
                def sc_tile_p2():
                    return psum_sc.tile(
                        [P, 2 * QCH], f32, tag="sc", name="sc2"
                    )

                def emit_outproj_tile(b, j, spare_psum=False):
                    m0 = b * S + j * P
                    for ec in range(E // 512):
                        esl = bass.ts(ec, 512)
                        if spare_psum and ec % 2 == 1:
                            # after the last pass the score banks are free
                            op = psum_sc.tile([P, 512], f32, tag="sc", name="op_s")
                        else:
                            op = psum_op.tile([P, 512], f32, tag="op", name="op")
                        nc.tensor.matmul(
                            op, YTC[:, bass.ds(m0, P)], woc_sb[:, esl],
                            start=True, stop=True,
                        )
                        osb = out_pool.tile([P, 512], f16, name="osb")
                        if spare_psum:
                            nc.scalar.copy(osb, op)   # ACT is idle at the tail
                        else:
                            nc.vector.tensor_copy(osb, op)
                        nc.sync.dma_start(out_d[bass.ds(m0, P), esl], osb)

                def emit_outproj(b, jlo, jhi, spare_psum=False):
                    for j in range(jlo, jhi):
                        emit_outproj_tile(b, j, spare_psum)

                # --- deferred batch-1 projection filler units ---
                def make_proj_units():
                    units = []
                    for mc in range(S // MCH, M // MCH):
                        msl = bass.ts(mc, MCH)
                        state = {}

                        def u_q(mc=mc, msl=msl, state=state):
                            xts = []
                            for kc in range(KC):
                                xt = xt_pool.tile(
                                    [P, MCH], f16, tag="xt2", bufs=18, name="xt2"
                                )
                                nc.sync.dma_start(xt, xT_d[bass.ts(kc, P), msl])
                                xts.append(xt)
                            state["xts"] = xts
                            qp = psum_op.tile([P, MCH], f32, tag="op", name="qp2")
                            for kc in range(KC):
                                nc.tensor.matmul(
                                    qp, wq_sb[:, kc, :], xts[kc],
                                    start=(kc == 0), stop=(kc == KC - 1),
                                )
                            nc.vector.tensor_scalar_add(QT[:, msl], qp, bq_sb)

                        def u_k(mc=mc, msl=msl, state=state):
                            kp = psum_op.tile([P, MCH], f32, tag="op", name="kp2")
                            for kc in range(KC):
                                nc.tensor.matmul(
                                    kp, wk_sb[:, kc, :], state["xts"][kc],
                                    start=(kc == 0), stop=(kc == KC - 1),
                                )
                            nc.vector.tensor_scalar_add(KT[:, msl], kp, bk_sb)

                        def u_v(mc=mc, msl=msl, state=state):
                            vp = psum_op.tile([P, MCH], f32, tag="op", name="vp2")
                            for kc in range(KC):
                                nc.tensor.matmul(
                                    vp, wv_sb[:, kc, :], state["xts"][kc],
                                    start=(kc == 0), stop=(kc == KC - 1),
                                )
                            vt = vt_pool.tile([P, MCH], f16, name="vt2", tag="vt2")
                            nc.vector.tensor_scalar_add(vt, vp, bv_sb)
                            state["vt"] = vt

                        def u_t(mc=mc, state=state):
                            vt = state["vt"]
                            for j in range(MCH // P):
                                mt = mc * (MCH // P) + j
                                vtp = psum_op.tile(
                                    [P, P], f16, tag="op", name="vtp2"
                                )
                                nc.tensor.transpose(vtp, vt[:, bass.ts(j, P)], ident)
                                copy_vtp(mt, vtp)

                        units += [u_q, u_k, u_v, u_t]
                    return units

                passes = [(b, qc) for b in range(B) for qc in range(NQC)][1:]
                pending = p1_pending
                filler = list(make_proj_units())
                for pi, (b, qc) in enumerate(passes):
                    if (b, qc) == (1, 0):
                        # deferred projections must be fully emitted before
                        # any batch-1 read (emission order defines dataflow)
                        while filler:
                            filler.pop(0)()
                        filler = [
                            (lambda b0=0, j0=j: emit_outproj_tile(b0, j0))
                            for j in range(S // P)
                        ]
                    av0 = psum_av.tile([P, QCH], f32, tag="av", name="av0")
                    av1 = psum_av.tile([P, QCH], f32, tag="av", name="av1")
                    for t in range(NKT):
                        emit_attn_tile(b, qc, t, av0, av1, sc_tile_p2, pt_tile)
                        # early-emit the previous pass's normalization (DVE/DMA
                        # only) so its reciprocal never gates later PE work
                        if t == 1 and pending:
                            bp, qcp, a0, a1 = pending.pop(0)
                            emit_norm(bp, qcp, a0, a1)
                            if bp == 1:
                                # batch-1 qcp columns are now normalized
                                filler.extend(
                                    (lambda b1=1, j1=j: emit_outproj_tile(b1, j1))
                                    for j in range(4 * qcp, 4 * qcp + 4)
                                )
                        # interleave independent PE work (deferred projections,
                        # ready out-proj tiles) to keep the PE saturated; slow
                        # the pop rate late so some filler remains to overlap
                        # the tail norm's DMA-bounce latency
                        pop_mod = 4 if (b, qc) >= (1, 2) else 2
                        if filler and t % pop_mod == pop_mod - 1:
                            filler.pop(0)()
                    stage_avs(b, qc, av0, av1, pending)
                # tail: emit the last norm FIRST (DVE/DMA only) so its DMA
                # bounce overlaps the remaining filler matmuls, then the
                # out-proj tiles it unblocks
                b_l, qc_l, a0_l, a1_l = pending.pop(0)   # (1,3)
                emit_norm(b_l, qc_l, a0_l, a1_l)
                while filler:
                    filler.pop(0)()
                emit_outproj(1, 12, 16, spare_psum=True)

    nc.compile()
    return nc


def kernel(x, mask, Wq, bq, Wk, bk, Wv, bv, Wo, bo):
    global LAST_RESULTS, _PROGRAM
    from concourse.bass_utils import run_bass_kernel_spmd

    if _PROGRAM is None:
        _PROGRAM = _build_program()
    nc = _PROGRAM

    f16 = np.float16
    x = np.asarray(x, dtype=np.float32)
    mask = np.asarray(mask)
    f32c = lambda a: np.ascontiguousarray(np.asarray(a, dtype=np.float32))

    xT = np.ascontiguousarray(x.reshape(M, E).T.astype(f16))     # [E, M]
    maskf = np.where(mask, np.float32(NEG), np.float32(0.0)).astype(np.float32)
    maskT = np.ascontiguousarray(
        maskf.reshape(B, 16, P).transpose(2, 0, 1).reshape(P, B * 16)
    )
    scale = np.float32(1.0 / np.sqrt(D))

    in_maps = []
    for c in range(NCORES):
        csl = slice(P * c, P * (c + 1))
        wq_c = (np.asarray(Wq, dtype=np.float32)[:, csl] * scale).astype(f16)
        wk_c = np.asarray(Wk, dtype=np.float32)[:, csl].astype(f16)
        wv_c = np.asarray(Wv, dtype=np.float32)[:, csl].astype(f16)
        in_maps.append(
            {
                "xT": xT,
                "wq": np.ascontiguousarray(wq_c.reshape(KC, P, P).transpose(1, 0, 2)),
                "wk": np.ascontiguousarray(wk_c.reshape(KC, P, P).transpose(1, 0, 2)),
                "wv": np.ascontiguousarray(wv_c.reshape(KC, P, P).transpose(1, 0, 2)),
                "woc": np.ascontiguousarray(
                    np.asarray(Wo, dtype=np.float32)[csl, :].astype(f16)
                ),
                "bq": f32c(np.asarray(bq)[csl] * scale).reshape(P, 1),
                "bk": f32c(np.asarray(bk)[csl]).reshape(P, 1),
                "bv": f32c(np.asarray(bv)[csl]).reshape(P, 1),
                "maskT": maskT,
            }
        )

    trace = bool(os.environ.get("KERNEL_TRACE"))
    LAST_RESULTS = run_bass_kernel_spmd(
        nc, in_maps, list(range(NCORES)), trace=trace
    )

    acc = np.zeros((M, E), dtype=np.float64)
    for res in LAST_RESULTS.results:
        acc += res["out"].astype(np.float64)
    out = (acc + np.asarray(bo, dtype=np.float64)[None, :]).astype(np.float32)
    return out.reshape(B, S, E)
